# revision 1
# baseline (speedup 1.0000x reference)
"""Trainium2 Bass kernel for nn_CheiralityLayer (cheirality loss) — v2.

Reference (per batch element b):
  gray = mean(img_pair[b, :3], axis=0)                       # [H, W]
  gx[h,w] = gray[h,w+1] - gray[h,w-1]   (zero padded)
  gy[h,w] = gray[h+1,w] - gray[h-1,w]   (zero padded)
  n = sqrt(gx^2 + gy^2 + 1e-8)
  P = gx*(x*V2 - V0) + gy*(y*V2 - V1)
  R = gx*(W0*x*y - W1*(x^2+1) + W2*y) + gy*(W0*(y^2+1) - W1*x*y - W2*x)
  rho = (P/n) * (nf0 + nf1 - R/n)
  out = mean(gelu(-rho))   (exact erf gelu)

v2 strategy (data parallel: 2 images/core, 4 row-bands of 120 rows each):
- gray is computed UNSCALED (c0+c1+c2, stationary entries 1.0); since grad
  dirs are normalized the 3x scale cancels exactly when eps -> 9e-8.
- gray (row-select) and gy (row stencil) via PE matmuls off the raw img
  rows with host-built banded bf16 stationaries (entries +-1, exact).
- gx via one 16-bit DVE column-shift subtract on an fp16 zero-padded
  graypad (partition start 0 - HW requires starts in {0,32,64,96}).
- squares + n2 on the otherwise idle Pool engine; 1/n2 fp32 on DVE;
  rinv = sqrt on ACT -> bf16.
- gx,gy normalized (bf16) BEFORE the P/I1/v contractions so PSUM holds
  P/n and v = nfs - R/n directly:
    P/n  = V2*xgxn - V0*gxn + (V2 y - V1) gyn
    I1   = -W1*xgxn + (W0 y) gxn + (-W1 y - W2) gyn
    tXn  = (-I1)*x      (ACT drains I1 with scale=-1, DVE multiplies by x)
    v    = (nf0+nf1) + tXn + (W1 - W2 y) gxn - W0(y^2+1) gyn
  and rho = (P/n) * v via one DVE multiply off the drained P.
- all diag/scalar stationaries are built ON-CHIP (iota mask times
  per-partition value column) from a tiny DMA'd table: the baseline's 4MB
  constants DMA (~12us) becomes ~100KB.
- per-tile erf-GELU(+accum) on ACT hides the reduction tail.
"""

import numpy as np

B, C, H, W = 16, 6, 480, 640
NCORES = 8
BPC = B // NCORES          # images per core
NPOS = 4                   # row bands per image
NT = BPC * NPOS            # tiles per core
TH = 120                   # output rows per band
LR = 124                   # loaded img rows per band (stencil halo)
RS = [0, 118, 238, 356]    # first loaded img row per band (clamped)
NSPLIT = [(0, 320), (320, 640)]
EPS = 9e-8                 # 9x reference eps (gray unscaled by 3)

NDIAG = BPC * NPOS * 5     # y-affine diag value columns
NSID = BPC * 3             # scalar-identity value columns
NVAL = NDIAG + NSID

F_B = 2 * NPOS * TH        # gmat1 + dmat2 columns (bf16 tensor)

_CACHE = {}


def _build_program(check_mode=False):
    """check_mode: skip the gelu+reduce tail (CoreSim lacks Gelu) and DMA
    per-pixel rho out as [TH, NT*W] f32 for numerical validation."""
    import concourse.bacc as bacc
    import concourse.tile as tile
    import concourse.mybir as mybir
    from contextlib import ExitStack

    f32 = mybir.dt.float32
    f32r = mybir.dt.float32r
    bf16 = mybir.dt.bfloat16
    f16 = mybir.dt.float16
    i16 = mybir.dt.int16
    AF = mybir.ActivationFunctionType
    OP = mybir.AluOpType

    nc = bacc.Bacc(
        "TRN2", target_bir_lowering=False, debug=False, enable_asserts=False
    )

    img_d = nc.dram_tensor("img3", [BPC, 3, H, W], f32r, kind="ExternalInput").ap()
    nf_d = nc.dram_tensor("nf", [BPC, 2, H, W], f32r, kind="ExternalInput").ap()
    cstb_d = nc.dram_tensor("cstb", [LR, F_B], f32r, kind="ExternalInput").ap()
    cstv_d = nc.dram_tensor("cstv", [128, NVAL], f32, kind="ExternalInput").ap()
    csth_d = nc.dram_tensor("csth", [128, W], f32, kind="ExternalInput").ap()
    if check_mode:
        rho_d = nc.dram_tensor(
            "rho_dbg", [TH, NT * W], f32, kind="ExternalOutput"
        ).ap()
    out_d = nc.dram_tensor("out", [1, 1], f32, kind="ExternalOutput").ap()

    def half(x):
        """[P, 640] AP -> [P, 2, 320] view matching psum halves."""
        return x.rearrange("p (b c) -> p b c", b=2)

    with tile.TileContext(nc) as tc, ExitStack() as ctx:
        consts = ctx.enter_context(tc.tile_pool(name="consts", bufs=1))
        imgp = ctx.enter_context(tc.tile_pool(name="imgp", bufs=3))
        nfp = ctx.enter_context(tc.tile_pool(name="nfp", bufs=2))
        work = ctx.enter_context(tc.tile_pool(name="work", bufs=2))
        psum = ctx.enter_context(tc.tile_pool(name="psum", bufs=1, space="PSUM"))

        cstb = consts.tile([LR, F_B], f32r)
        nc.sync.dma_start(cstb, cstb_d)
        cstv = consts.tile([128, NVAL], f32)
        nc.sync.dma_start(cstv, cstv_d)
        csth = consts.tile([128, W], f32)
        nc.sync.dma_start(csth, csth_d)
        X16 = consts.tile([128, W], f16)
        nc.vector.tensor_copy(X16, csth)
        X = X16[0:TH, :]

        def gmat(p):
            return cstb[0:LR, p * TH : (p + 1) * TH]

        def dmat(p):
            return cstb[0:LR, (NPOS + p) * TH : (NPOS + p + 1) * TH]

        # identity mask: iota(i - p) == 0
        io16 = consts.tile([TH, TH], i16)
        nc.gpsimd.iota(io16, [[1, TH]], base=0, channel_multiplier=-1)
        mask = consts.tile([TH, TH], bf16)
        nc.vector.tensor_scalar(mask, io16, 0, None, OP.is_equal)

        # on-chip diag/sid stationaries from per-partition value columns
        dgt = consts.tile([TH, NDIAG * TH], bf16)
        for col in range(NDIAG):
            nc.vector.tensor_scalar_mul(
                dgt[:, col * TH : (col + 1) * TH], mask, cstv[0:TH, col : col + 1]
            )
        sidt = consts.tile([TH, NSID * TH], bf16)
        for s in range(NSID):
            nc.vector.tensor_scalar_mul(
                sidt[:, s * TH : (s + 1) * TH],
                mask,
                cstv[0:TH, NDIAG + s : NDIAG + s + 1],
            )

        def dg(i, p, k):
            col = ((i * NPOS) + p) * 5 + k
            return dgt[0:TH, col * TH : (col + 1) * TH]

        def sid(i, k):
            s = i * 3 + k
            return sidt[0:TH, s * TH : (s + 1) * TH]

        acc = consts.tile([128, NT], f32)
        nc.vector.memset(acc, 0.0)
        ones_t = consts.tile([128, 1], f32)
        nc.vector.memset(ones_t, 1.0)

        rho_all = consts.tile([128, NT * W], bf16)

        def front(t):
            i, p = divmod(t, NPOS)
            imgt = imgp.tile([LR, 3, W], f32r, tag="imgt")
            nc.sync.dma_start(
                imgt,
                img_d[i, :, RS[p] : RS[p] + LR, :].rearrange("c h w -> h c w"),
            )
            nft = nfp.tile([TH, 2, W], f32r, tag="nft")
            nc.sync.dma_start(
                nft,
                nf_d[i, :, TH * p : TH * (p + 1), :].rearrange("c h w -> h c w"),
            )

            # gray (unscaled channel sum, row-select) on PE
            gray_ps = psum.tile([TH, 2, 512], f32, tag="gray")
            for c3 in range(3):
                for b, (n0, n1) in enumerate(NSPLIT):
                    nc.tensor.matmul(
                        gray_ps[:, b, 0:320],
                        gmat(p),
                        imgt[:, c3, n0:n1],
                        start=(c3 == 0),
                        stop=(c3 == 2),
                    )
            graypad = work.tile([TH, W + 2], f16, tag="graypad")
            nc.gpsimd.memset(graypad[:, 0:1], 0.0)
            nc.gpsimd.memset(graypad[:, W + 1 : W + 2], 0.0)
            nc.scalar.copy(half(graypad[:, 1 : W + 1]), gray_ps[:, :, 0:320])

            # gy (row stencil) on PE off the raw img rows
            gy_ps = psum.tile([TH, 2, 512], f32, tag="gy")
            for c3 in range(3):
                for b, (n0, n1) in enumerate(NSPLIT):
                    nc.tensor.matmul(
                        gy_ps[:, b, 0:320],
                        dmat(p),
                        imgt[:, c3, n0:n1],
                        start=(c3 == 0),
                        stop=(c3 == 2),
                    )
            gy = work.tile([TH, W], f16, tag="gy")
            nc.scalar.copy(half(gy), gy_ps[:, :, 0:320])
            gy2 = work.tile([TH, W], f32, tag="gy2")
            nc.gpsimd.tensor_mul(gy2, gy, gy)

            # gx: 16-bit column-shift subtract (partition start 0)
            gx = work.tile([TH, W], f16, tag="gx")
            nc.vector.tensor_sub(gx, graypad[:, 2 : W + 2], graypad[:, 0:W])
            gx2 = work.tile([TH, W], f32, tag="gx2")
            nc.gpsimd.tensor_mul(gx2, gx, gx)

            n2 = work.tile([TH, W], f32, tag="n2")
            nc.vector.scalar_tensor_tensor(n2, gx2, EPS, gy2, OP.add, OP.add)
            inv2 = work.tile([TH, W], f32, tag="inv2")
            nc.vector.reciprocal_approx_fast(out=inv2, in_=n2)
            rinv = work.tile([TH, W], bf16, tag="rinv")
            nc.scalar.sqrt(rinv, inv2)
            return (t, i, p, nft, gx, gy, rinv)

        def back1(st):
            t, i, p, nft, gx, gy, rinv = st
            gxn = work.tile([TH, W], bf16, tag="gxn")
            nc.vector.tensor_mul(gxn, gx, rinv)
            gyn = work.tile([TH, W], bf16, tag="gyn")
            nc.vector.tensor_mul(gyn, gy, rinv)
            xgxn = work.tile([TH, W], bf16, tag="xgxn")
            nc.vector.tensor_mul(xgxn, gxn, X)
            nfs = work.tile([TH, W], bf16, tag="nfs")
            nc.gpsimd.tensor_add(nfs, nft[:, 0, :], nft[:, 1, :])

            # P/n = V2*xgxn - V0*gxn + (V2 y - V1) gyn
            P_ps = psum.tile([TH, 2, 512], f32, tag="pv")
            psrc = [(sid(i, 0), xgxn), (sid(i, 1), gxn), (dg(i, p, 0), gyn)]
            for k, (m, src) in enumerate(psrc):
                for b, (n0, n1) in enumerate(NSPLIT):
                    nc.tensor.matmul(
                        P_ps[:, b, 0:320],
                        m,
                        src[:, n0:n1],
                        start=(k == 0),
                        stop=(k == len(psrc) - 1),
                    )

            # I1 = -W1*xgxn + (W0 y) gxn + (-W1 y - W2) gyn
            I1_ps = psum.tile([TH, 2, 512], f32, tag="i1")
            isrc = [(sid(i, 2), xgxn), (dg(i, p, 1), gxn), (dg(i, p, 2), gyn)]
            for k, (m, src) in enumerate(isrc):
                for b, (n0, n1) in enumerate(NSPLIT):
                    nc.tensor.matmul(
                        I1_ps[:, b, 0:320],
                        m,
                        src[:, n0:n1],
                        start=(k == 0),
                        stop=(k == len(isrc) - 1),
                    )
            return (t, i, p, P_ps, I1_ps, gxn, gyn, nfs)

        def back2(st2):
            t, i, p, P_ps, I1_ps, gxn, gyn, nfs = st2
            I1b = work.tile([TH, W], bf16, tag="i1b")
            nc.scalar.activation(half(I1b), I1_ps[:, :, 0:320], AF.Copy, scale=-1.0)
            tXn = work.tile([TH, W], bf16, tag="txn")
            nc.vector.tensor_mul(tXn, I1b, X)

            Pb = work.tile([TH, W], bf16, tag="pb")
            nc.scalar.copy(half(Pb), P_ps[:, :, 0:320])

            # v = nfs + tXn + (W1 - W2 y) gxn - W0(y^2+1) gyn (reuses P banks)
            v_ps = psum.tile([TH, 2, 512], f32, tag="pv")
            vsrc = [
                (mask, nfs),
                (mask, tXn),
                (dg(i, p, 3), gxn),
                (dg(i, p, 4), gyn),
            ]
            for k, (m, src) in enumerate(vsrc):
                for b, (n0, n1) in enumerate(NSPLIT):
                    nc.tensor.matmul(
                        v_ps[:, b, 0:320],
                        m,
                        src[:, n0:n1],
                        start=(k == 0),
                        stop=(k == len(vsrc) - 1),
                    )

            rho = rho_all[0:TH, t * W : (t + 1) * W]
            nc.vector.tensor_mul(half(rho), half(Pb), v_ps[:, :, 0:320])
            if check_mode:
                rho32 = work.tile([TH, W], f32, tag="rho32")
                nc.vector.tensor_copy(rho32, rho)
                nc.sync.dma_start(rho_d[:, t * W : (t + 1) * W], rho32)

        st = front(0)
        for t in range(NT):
            st2 = back1(st)
            nst = front(t + 1) if t + 1 < NT else None
            back2(st2)
            st = nst

        gelu_out = consts.tile([128, NT * W], bf16)
        if not check_mode:
            nc.scalar.activation(
                gelu_out[0:TH, :],
                rho_all[0:TH, :],
                AF.Gelu,
                scale=-1.0,
                accum_out=acc[0:TH, 0:1],
            )
        accs = consts.tile([128, 1], f32)
        nc.vector.reduce_sum(
            accs[0:TH, :], acc[0:TH, 0:NT], axis=mybir.AxisListType.X
        )
        out_ps = psum.tile([1, 1], f32, tag="gray")
        nc.tensor.matmul(
            out_ps, accs[0:TH, :], ones_t[0:TH, :], start=True, stop=True
        )
        res = consts.tile([1, 1], f32)
        nc.scalar.copy(res, out_ps)
        nc.sync.dma_start(out_d, res)

    nc.compile()
    return nc


def _host_constants(pose_np):
    """Per-core host-built constants.

    Returns (cstb, cstv_list, csth): cstb/csth shared, cstv per core."""
    # gmat1: img row (RS[p]+k) -> gray row (120p + j), entries 1.0
    # dmat2: gy[j] = gray[120p+j+1] - gray[120p+j-1] (zero padded rows)
    gmat1 = np.zeros((LR, NPOS, TH), np.float32)
    dmat2 = np.zeros((LR, NPOS, TH), np.float32)
    for p in range(NPOS):
        for j in range(TH):
            row = TH * p + j
            gmat1[row - RS[p], p, j] = 1.0
            if row + 1 <= H - 1:
                dmat2[row + 1 - RS[p], p, j] += 1.0
            if row - 1 >= 0:
                dmat2[row - 1 - RS[p], p, j] -= 1.0
    cstb = np.concatenate(
        [gmat1.reshape(LR, -1), dmat2.reshape(LR, -1)], axis=1
    )

    csth = np.broadcast_to(np.arange(W, dtype=np.float32), (128, W)).copy()

    cstv_list = []
    for core in range(NCORES):
        vals = np.zeros((128, NVAL), np.float32)
        for i in range(BPC):
            b = core * BPC + i
            V0, V1, V2, W0, W1, W2 = [float(x) for x in pose_np[b]]
            for p in range(NPOS):
                yv = (TH * p + np.arange(TH)).astype(np.float32)
                base = ((i * NPOS) + p) * 5
                vals[0:TH, base + 0] = V2 * yv - V1
                vals[0:TH, base + 1] = W0 * yv
                vals[0:TH, base + 2] = -W1 * yv - W2
                vals[0:TH, base + 3] = W1 - W2 * yv
                vals[0:TH, base + 4] = -W0 * (yv * yv + 1.0)
            vals[0:TH, NDIAG + i * 3 + 0] = V2
            vals[0:TH, NDIAG + i * 3 + 1] = -V0
            vals[0:TH, NDIAG + i * 3 + 2] = -W1
        cstv_list.append(vals)
    return cstb, cstv_list, csth


def kernel(img_pair, pose, normal_flow):
    from concourse.bass_utils import run_bass_kernel_spmd

    img_pair = np.asarray(img_pair, dtype=np.float32)
    pose = np.asarray(pose, dtype=np.float32)
    normal_flow = np.asarray(normal_flow, dtype=np.float32)

    if "nc" not in _CACHE:
        _CACHE["nc"] = _build_program()
    nc = _CACHE["nc"]

    cstb, cstv_list, csth = _host_constants(pose)
    in_maps = []
    for core in range(NCORES):
        b0 = core * BPC
        in_maps.append(
            {
                "img3": np.ascontiguousarray(img_pair[b0 : b0 + BPC, :3]),
                "nf": np.ascontiguousarray(normal_flow[b0 : b0 + BPC]),
                "cstb": cstb,
                "cstv": cstv_list[core],
                "csth": csth,
            }
        )

    _CACHE["in_maps"] = in_maps
    res = run_bass_kernel_spmd(nc, in_maps, core_ids=list(range(NCORES)))
    total = np.float64(0.0)
    for r in res.results:
        total += np.float64(r["out"][0, 0])
    out = np.float32(total / (B * H * W))
    return np.asarray(out, dtype=np.float32)



# revision 2
# speedup vs baseline: 1.1321x; 1.1321x over previous
"""Trainium2 Bass kernel for nn_CheiralityLayer (cheirality loss) — v3.

Reference (per batch element b):
  gray = mean(img_pair[b, :3], axis=0)                       # [H, W]
  gx[h,w] = gray[h,w+1] - gray[h,w-1]   (zero padded)
  gy[h,w] = gray[h+1,w] - gray[h-1,w]   (zero padded)
  n = sqrt(gx^2 + gy^2 + 1e-8)
  P = gx*(x*V2 - V0) + gy*(y*V2 - V1)
  R = gx*(W0*x*y - W1*(x^2+1) + W2*y) + gy*(W0*(y^2+1) - W1*x*y - W2*x)
  rho = (P/n) * (nf0 + nf1 - R/n)
  out = mean(gelu(-rho))   (exact erf gelu)

v3 strategy (2 images/core, 4 row-bands of 120 rows each), engine-balanced
so every engine stays under the ~4.3us/tile DMA rate:
- gray_ext (122 rows: band + 1-row halo each side, halo rows in partitions
  120/121 so gx reads start at partition 0) via 6 f32r PE matmuls; single
  ACT drain to f16 graypad.
- gy via 2 f16 PE matmuls off graypad (cheaper than 6 f32r off raw img).
- gx (column stencil) + gx^2 on the otherwise idle Pool engine.
- gy^2 via ACT Square directly from PSUM (no gy drain); 1/n via ACT
  Abs_reciprocal_sqrt in ONE op (no DVE reciprocal); both live in the same
  act table as Copy/Square so the body needs a single table load.
- I1/tXn intermediate of v2 replaced by explicit x2gxn/xgyn products:
  drops the extra PSUM group (8 banks exactly) and a PSUM round-trip.
- v group absorbs nf0+nf1 via two identity-stationary f32r passes (PE has
  slack; saves a 1.4us Pool add).
- rho = Pdrain(ACT) * v_psum on DVE; per-core gelu+reduce tail as in v2.
- y-affine diag stationaries: image-0 set built on DVE during the initial
  DMA window; image-1 set prebuilt on host and DMA'd (overlapped).
"""

import numpy as np

B, C, H, W = 16, 6, 480, 640
NCORES = 8
BPC = B // NCORES          # images per core
NPOS = 4                   # row bands per image
NT = BPC * NPOS            # tiles per core
TH = 120                   # output rows per band
LR = 122                   # loaded img rows per band (gray rows -1..120)
RS = [0, 119, 239, 358]    # first loaded img row per band (clamped)
NSPLIT = [(0, 320), (320, 640)]
EPS = 9e-8                 # 9x reference eps (gray unscaled by 3)

NBLK = 23                  # const blocks per image: 4 bands x 5 diag + 3 sid
USE_ARS = True             # Abs_reciprocal_sqrt on ACT vs recip(DVE)+sqrt

_CACHE = {}


def _build_program(check_mode=False):
    """check_mode: skip the gelu+reduce tail (CoreSim lacks Gelu) and DMA
    per-pixel rho out as [TH, NT*W] f32 for numerical validation."""
    import concourse.bacc as bacc
    import concourse.tile as tile
    import concourse.mybir as mybir
    from contextlib import ExitStack

    f32 = mybir.dt.float32
    f32r = mybir.dt.float32r
    bf16 = mybir.dt.bfloat16
    f16 = mybir.dt.float16
    i16 = mybir.dt.int16
    AF = mybir.ActivationFunctionType
    OP = mybir.AluOpType

    nc = bacc.Bacc(
        "TRN2", target_bir_lowering=False, debug=False, enable_asserts=False
    )

    img_d = nc.dram_tensor("img3", [BPC, 3, H, W], f32r, kind="ExternalInput").ap()
    nf_d = nc.dram_tensor("nf", [BPC, 2, H, W], f32r, kind="ExternalInput").ap()
    cstb_d = nc.dram_tensor("cstb", [LR, NPOS * LR], f32r, kind="ExternalInput").ap()
    cstd_d = nc.dram_tensor("cstd", [LR, NPOS * TH], f16, kind="ExternalInput").ap()
    cstv_d = nc.dram_tensor("cstv", [128, 2 * NBLK], f32, kind="ExternalInput").ap()
    dgt1_d = nc.dram_tensor("dgt1", [TH, NBLK * TH], bf16, kind="ExternalInput").ap()
    if check_mode:
        rho_d = nc.dram_tensor(
            "rho_dbg", [TH, NT * W], f32, kind="ExternalOutput"
        ).ap()
    out_d = nc.dram_tensor("out", [1, 1], f32, kind="ExternalOutput").ap()

    def half(x):
        """[P, 640] AP -> [P, 2, 320] view matching psum halves."""
        return x.rearrange("p (b c) -> p b c", b=2)

    with tile.TileContext(nc) as tc, ExitStack() as ctx:
        consts = ctx.enter_context(tc.tile_pool(name="consts", bufs=1))
        imgp = ctx.enter_context(tc.tile_pool(name="imgp", bufs=3))
        nfp = ctx.enter_context(tc.tile_pool(name="nfp", bufs=3))
        work = ctx.enter_context(tc.tile_pool(name="work", bufs=2))
        psum = ctx.enter_context(tc.tile_pool(name="psum", bufs=1, space="PSUM"))

        # --- small constants first on the DMA queue ---
        cstb = consts.tile([LR, NPOS * LR], f32r)
        nc.sync.dma_start(cstb, cstb_d)
        cstd = consts.tile([LR, NPOS * TH], f16)
        nc.sync.dma_start(cstd, cstd_d)
        cstv = consts.tile([128, 2 * NBLK], f32)
        nc.sync.dma_start(cstv, cstv_d)

        def gmat(p):
            return cstb[0:LR, p * LR : (p + 1) * LR]

        def dmat(p):
            return cstd[0:LR, p * TH : (p + 1) * TH]

        # --- on-chip builds (overlap the initial DMA window) ---
        # x-coordinate row (0..639) on every partition, f16 (exact ints)
        xio = consts.tile([128, W], i16)
        nc.gpsimd.iota(xio, [[1, W]], base=0, channel_multiplier=0)
        X16 = consts.tile([128, W], f16)
        nc.vector.tensor_copy(X16, xio)
        X = X16[0:TH, :]

        # identity mask: iota(i - p) == 0
        io16 = consts.tile([TH, TH], i16)
        nc.gpsimd.iota(io16, [[1, TH]], base=0, channel_multiplier=-1)
        mask = consts.tile([TH, TH], bf16)
        nc.vector.tensor_scalar(mask, io16, 0, None, OP.is_equal)
        mask32 = consts.tile([TH, TH], f32r)
        nc.vector.tensor_copy(mask32, mask)

        # diag/sid stationaries: image 0 built on DVE, image 1 DMA'd below
        dgt = consts.tile([TH, 2 * NBLK * TH], bf16)
        for blk in range(NBLK):
            nc.vector.tensor_scalar_mul(
                dgt[:, blk * TH : (blk + 1) * TH], mask, cstv[0:TH, blk : blk + 1]
            )

        def dg(i, p, k):
            blk = i * NBLK + p * 5 + k
            return dgt[0:TH, blk * TH : (blk + 1) * TH]

        def sid(i, s):
            blk = i * NBLK + 20 + s
            return dgt[0:TH, blk * TH : (blk + 1) * TH]

        acc = consts.tile([128, 1], f32)
        nc.vector.memset(acc, 0.0)
        ones_t = consts.tile([128, 1], f32)
        nc.vector.memset(ones_t, 1.0)

        rho_all = consts.tile([TH, NT * W], bf16)
        gelu_out = consts.tile([TH, NT * W], bf16)

        def front(t):
            i, p = divmod(t, NPOS)
            imgt = imgp.tile([LR, 3, W], f32r, tag="imgt")
            nc.sync.dma_start(
                imgt,
                img_d[i, :, RS[p] : RS[p] + LR, :].rearrange("c h w -> h c w"),
            )
            nft = nfp.tile([TH, 2, W], f32r, tag="nft")
            nc.sync.dma_start(
                nft,
                nf_d[i, :, TH * p : TH * (p + 1), :].rearrange("c h w -> h c w"),
            )
            return (imgt, nft)

        def grayblock(t, ft):
            imgt, nft = ft
            i, p = divmod(t, NPOS)
            # gray_ext (unscaled channel sum; halo rows at partitions 120/121)
            gray_ps = psum.tile([LR, 2, 512], f32, tag="gray")
            for c3 in range(3):
                for b, (n0, n1) in enumerate(NSPLIT):
                    nc.tensor.matmul(
                        gray_ps[:, b, 0:320],
                        gmat(p),
                        imgt[:, c3, n0:n1],
                        start=(c3 == 0),
                        stop=(c3 == 2),
                    )
            graypad = work.tile([LR, W + 2], f16, tag="graypad")
            nc.gpsimd.memset(graypad[:, 0:1], 0.0)
            nc.gpsimd.memset(graypad[:, W + 1 : W + 2], 0.0)
            nc.scalar.copy(half(graypad[:, 1 : W + 1]), gray_ps[:, :, 0:320])
            return (imgt, nft, graypad)

        def gyblock(t, gt):
            imgt, nft, graypad = gt
            i, p = divmod(t, NPOS)
            # gy (row stencil) on PE off f16 graypad
            gy_ps = psum.tile([TH, 2, 512], f32, tag="gy")
            for b, (n0, n1) in enumerate(NSPLIT):
                nc.tensor.matmul(
                    gy_ps[:, b, 0:320],
                    dmat(p),
                    graypad[:, 1 + n0 : 1 + n1],
                    start=True,
                    stop=True,
                )
            # gx: column-shift subtract + square on Pool
            gx = work.tile([TH, W], f16, tag="gx")
            nc.gpsimd.tensor_sub(
                gx, graypad[0:TH, 2 : W + 2], graypad[0:TH, 0:W]
            )
            gx2 = work.tile([TH, W], bf16, tag="gx2")
            nc.gpsimd.tensor_mul(gx2, gx, gx)
            # gy^2 straight off PSUM on ACT
            gy2 = work.tile([TH, W], bf16, tag="gy2")
            nc.scalar.activation(half(gy2), gy_ps[:, :, 0:320], AF.Square)
            n2 = work.tile([TH, W], bf16, tag="n2")
            nc.vector.scalar_tensor_tensor(n2, gx2, EPS, gy2, OP.add, OP.add)
            rinv = work.tile([TH, W], bf16, tag="rinv")
            if USE_ARS:
                nc.scalar.activation(rinv, n2, AF.Abs_reciprocal_sqrt)
            else:
                n2f = work.tile([TH, W], f32, tag="n2f")
                nc.vector.tensor_copy(n2f, n2)
                inv2 = work.tile([TH, W], f32, tag="inv2")
                nc.vector.reciprocal_approx_fast(out=inv2, in_=n2f)
                nc.scalar.sqrt(rinv, inv2)
            # normalized products
            gxn = work.tile([TH, W], bf16, tag="gxn")
            nc.vector.tensor_mul(gxn, gx, rinv)
            gyn = work.tile([TH, W], bf16, tag="gyn")
            nc.vector.tensor_mul(half(gyn), gy_ps[:, :, 0:320], half(rinv))
            xgxn = work.tile([TH, W], bf16, tag="xgxn")
            nc.vector.tensor_mul(xgxn, gxn, X)
            xgyn = work.tile([TH, W], bf16, tag="xgyn")
            nc.vector.tensor_mul(xgyn, gyn, X)
            x2gxn = work.tile([TH, W], bf16, tag="x2gxn")
            nc.vector.tensor_mul(x2gxn, xgxn, X)
            return (t, i, p, nft, gxn, gyn, xgxn, xgyn, x2gxn)

        def backblock(st):
            t, i, p, nft, gxn, gyn, xgxn, xgyn, x2gxn = st
            # P/n = V2*xgxn - V0*gxn + (V2 y - V1) gyn
            P_ps = psum.tile([TH, 2, 512], f32, tag="P")
            psrc = [(sid(i, 0), xgxn), (sid(i, 1), gxn), (dg(i, p, 0), gyn)]
            for k, (m, src) in enumerate(psrc):
                for b, (n0, n1) in enumerate(NSPLIT):
                    nc.tensor.matmul(
                        P_ps[:, b, 0:320],
                        m,
                        src[:, n0:n1],
                        start=(k == 0),
                        stop=(k == len(psrc) - 1),
                    )
            # v = nf0 + nf1 - R/n
            #   = nf0 + nf1 + W1*x2gxn - W0y*xgxn + (W1y+W2)*xgyn
            #     + (W1 - W2y)*gxn - W0(y^2+1)*gyn
            v_ps = psum.tile([TH, 2, 512], f32, tag="V")
            vsrc = [
                (mask32, nft[:, 0, :]),
                (mask32, nft[:, 1, :]),
                (sid(i, 2), x2gxn),
                (dg(i, p, 1), xgxn),
                (dg(i, p, 2), xgyn),
                (dg(i, p, 3), gxn),
                (dg(i, p, 4), gyn),
            ]
            for k, (m, src) in enumerate(vsrc):
                for b, (n0, n1) in enumerate(NSPLIT):
                    nc.tensor.matmul(
                        v_ps[:, b, 0:320],
                        m,
                        src[:, n0:n1],
                        start=(k == 0),
                        stop=(k == len(vsrc) - 1),
                    )
            Pb = work.tile([TH, W], bf16, tag="pb")
            nc.scalar.copy(half(Pb), P_ps[:, :, 0:320])
            rho = rho_all[0:TH, t * W : (t + 1) * W]
            nc.vector.tensor_mul(half(rho), half(Pb), v_ps[:, :, 0:320])
            if check_mode:
                rho32 = work.tile([TH, W], f32, tag="rho32")
                nc.vector.tensor_copy(rho32, rho)
                nc.sync.dma_start(rho_d[:, t * W : (t + 1) * W], rho32)

        # --- software pipeline ---
        ft0 = front(0)
        ft1 = front(1)
        # image-1 stationaries arrive behind the first two tiles' data
        nc.sync.dma_start(dgt[:, NBLK * TH : 2 * NBLK * TH], dgt1_d)
        gt = grayblock(0, ft0)
        st = gyblock(0, gt)
        fts = {1: ft1}
        for k in range(NT):
            if k + 2 < NT:
                fts[k + 2] = front(k + 2)
            gt = grayblock(k + 1, fts.pop(k + 1)) if k + 1 < NT else None
            backblock(st)
            st = gyblock(k + 1, gt) if k + 1 < NT else None

        # --- tail: gelu + reduce ---
        if not check_mode:
            nc.scalar.activation(
                gelu_out[0:TH, :],
                rho_all[0:TH, :],
                AF.Gelu,
                scale=-1.0,
                accum_out=acc[0:TH, 0:1],
            )
        out_ps = psum.tile([1, 1], f32, tag="gray")
        nc.tensor.matmul(
            out_ps, acc[0:TH, 0:1], ones_t[0:TH, :], start=True, stop=True
        )
        res = consts.tile([1, 1], f32)
        nc.scalar.copy(res, out_ps)
        nc.sync.dma_start(out_d, res)

    nc.compile()
    return nc


def _host_constants(pose_np):
    """Host-built constants. cstb/cstd shared; cstv/dgt1 per core."""
    import ml_dtypes

    # gmat_ext: img row -> gray_ext partition (cols 0..119 = band rows,
    # col 120 = halo row band_start-1, col 121 = halo row band_start+120)
    gmat = np.zeros((LR, NPOS, LR), np.float32)
    # dmat: graypad partition -> gy row: gy[j] = gray[j+1] - gray[j-1]
    dmat = np.zeros((LR, NPOS, TH), np.float32)
    for p in range(NPOS):
        for j in range(LR):
            g = TH * p + j if j < TH else (TH * p - 1 if j == TH else TH * p + TH)
            if 0 <= g <= H - 1:
                gmat[g - RS[p], p, j] = 1.0
        for j in range(TH):
            q_plus = j + 1 if j < TH - 1 else LR - 1
            q_minus = j - 1 if j >= 1 else TH
            dmat[q_plus, p, j] += 1.0
            dmat[q_minus, p, j] -= 1.0
    cstb = gmat.reshape(LR, -1)
    cstd = dmat.reshape(LR, -1).astype(np.float16)

    cstv_list = []
    dgt1_list = []
    jj = np.arange(TH, dtype=np.float32)
    for core in range(NCORES):
        vals = np.zeros((128, 2 * NBLK), np.float32)
        for i in range(BPC):
            b = core * BPC + i
            V0, V1, V2, W0, W1, W2 = [float(x) for x in pose_np[b]]
            base = i * NBLK
            for p in range(NPOS):
                yv = TH * p + jj
                o = base + p * 5
                vals[0:TH, o + 0] = V2 * yv - V1
                vals[0:TH, o + 1] = -W0 * yv
                vals[0:TH, o + 2] = W1 * yv + W2
                vals[0:TH, o + 3] = W1 - W2 * yv
                vals[0:TH, o + 4] = -W0 * (yv * yv + 1.0)
            vals[0:TH, base + 20] = V2
            vals[0:TH, base + 21] = -V0
            vals[0:TH, base + 22] = W1
        cstv_list.append(vals)
        dgt1 = np.zeros((TH, NBLK * TH), np.float32)
        for blk in range(NBLK):
            np.fill_diagonal(
                dgt1[:, blk * TH : (blk + 1) * TH], vals[0:TH, NBLK + blk]
            )
        dgt1_list.append(dgt1.astype(ml_dtypes.bfloat16))
    return cstb, cstd, cstv_list, dgt1_list


def kernel(img_pair, pose, normal_flow):
    from concourse.bass_utils import run_bass_kernel_spmd

    img_pair = np.asarray(img_pair, dtype=np.float32)
    pose = np.asarray(pose, dtype=np.float32)
    normal_flow = np.asarray(normal_flow, dtype=np.float32)

    if "nc" not in _CACHE:
        _CACHE["nc"] = _build_program()
    nc = _CACHE["nc"]

    cstb, cstd, cstv_list, dgt1_list = _host_constants(pose)
    in_maps = []
    for core in range(NCORES):
        b0 = core * BPC
        in_maps.append(
            {
                "img3": np.ascontiguousarray(img_pair[b0 : b0 + BPC, :3]),
                "nf": np.ascontiguousarray(normal_flow[b0 : b0 + BPC]),
                "cstb": cstb,
                "cstd": cstd,
                "cstv": cstv_list[core],
                "dgt1": dgt1_list[core],
            }
        )

    _CACHE["in_maps"] = in_maps
    res = run_bass_kernel_spmd(nc, in_maps, core_ids=list(range(NCORES)))
    total = np.float64(0.0)
    for r in res.results:
        total += np.float64(r["out"][0, 0])
    out = np.float32(total / (B * H * W))
    return np.asarray(out, dtype=np.float32)


# revision 5
# speedup vs baseline: 1.1871x; 1.0486x over previous
"""Trainium2 Bass kernel for nn_CheiralityLayer (cheirality loss) — v3.

Reference (per batch element b):
  gray = mean(img_pair[b, :3], axis=0)                       # [H, W]
  gx[h,w] = gray[h,w+1] - gray[h,w-1]   (zero padded)
  gy[h,w] = gray[h+1,w] - gray[h-1,w]   (zero padded)
  n = sqrt(gx^2 + gy^2 + 1e-8)
  P = gx*(x*V2 - V0) + gy*(y*V2 - V1)
  R = gx*(W0*x*y - W1*(x^2+1) + W2*y) + gy*(W0*(y^2+1) - W1*x*y - W2*x)
  rho = (P/n) * (nf0 + nf1 - R/n)
  out = mean(gelu(-rho))   (exact erf gelu)

v3 strategy (2 images/core, 4 row-bands of 120 rows each), engine-balanced
so every engine stays under the ~4.3us/tile DMA rate:
- gray_ext (122 rows: band + 1-row halo each side, halo rows in partitions
  120/121 so gx reads start at partition 0) via 6 f32r PE matmuls; single
  ACT drain to f16 graypad.
- gy via 2 f16 PE matmuls off graypad (cheaper than 6 f32r off raw img).
- gx (column stencil) + gx^2 on the otherwise idle Pool engine.
- gy^2 via ACT Square directly from PSUM (no gy drain); 1/n via ACT
  Abs_reciprocal_sqrt in ONE op (no DVE reciprocal); both live in the same
  act table as Copy/Square so the body needs a single table load.
- I1/tXn intermediate of v2 replaced by explicit x2gxn/xgyn products:
  drops the extra PSUM group (8 banks exactly) and a PSUM round-trip.
- v group absorbs nf0+nf1 via two identity-stationary f32r passes (PE has
  slack; saves a 1.4us Pool add).
- rho = Pdrain(ACT) * v_psum on DVE; per-core gelu+reduce tail as in v2.
- y-affine diag stationaries: image-0 set built on DVE during the initial
  DMA window; image-1 set prebuilt on host and DMA'd (overlapped).
"""

import numpy as np

B, C, H, W = 16, 6, 480, 640
NCORES = 8
BPC = B // NCORES          # images per core
NPOS = 4                   # row bands per image
NT = BPC * NPOS            # tiles per core
TH = 120                   # output rows per band
LR = 122                   # loaded img rows per band (gray rows -1..120)
RS = [0, 119, 239, 358]    # first loaded img row per band (clamped)
NSPLIT = [(0, 320), (320, 640)]
EPS = 9e-8                 # 9x reference eps (gray unscaled by 3)

NBLK = 23                  # const blocks per image: 4 bands x 5 diag + 3 sid
USE_ARS = True             # Abs_reciprocal_sqrt on ACT vs recip(DVE)+sqrt

_CACHE = {}


def _build_program(check_mode=False):
    """check_mode: skip the gelu+reduce tail (CoreSim lacks Gelu) and DMA
    per-pixel rho out as [TH, NT*W] f32 for numerical validation."""
    import concourse.bacc as bacc
    import concourse.tile as tile
    import concourse.mybir as mybir
    from contextlib import ExitStack

    f32 = mybir.dt.float32
    f32r = mybir.dt.float32r
    bf16 = mybir.dt.bfloat16
    f16 = mybir.dt.float16
    i16 = mybir.dt.int16
    AF = mybir.ActivationFunctionType
    OP = mybir.AluOpType

    nc = bacc.Bacc(
        "TRN2", target_bir_lowering=False, debug=False, enable_asserts=False
    )

    img_d = nc.dram_tensor("img3", [BPC, 3, H, W], f32r, kind="ExternalInput").ap()
    nf_d = nc.dram_tensor("nf", [BPC, 2, H, W], f32r, kind="ExternalInput").ap()
    cstb_d = nc.dram_tensor("cstb", [LR, NPOS * LR], f32r, kind="ExternalInput").ap()
    cstd_d = nc.dram_tensor("cstd", [LR, NPOS * TH], f16, kind="ExternalInput").ap()
    cstv_d = nc.dram_tensor("cstv", [128, 2 * NBLK], f32, kind="ExternalInput").ap()
    dgt1_d = nc.dram_tensor("dgt1", [TH, NBLK * TH], bf16, kind="ExternalInput").ap()
    if check_mode:
        rho_d = nc.dram_tensor(
            "rho_dbg", [TH, NT * W], f32, kind="ExternalOutput"
        ).ap()
    out_d = nc.dram_tensor("out", [1, 1], f32, kind="ExternalOutput").ap()

    def half(x):
        """[P, 640] AP -> [P, 2, 320] view matching psum halves."""
        return x.rearrange("p (b c) -> p b c", b=2)

    with tile.TileContext(nc) as tc, ExitStack() as ctx:
        consts = ctx.enter_context(tc.tile_pool(name="consts", bufs=1))
        imgp = ctx.enter_context(tc.tile_pool(name="imgp", bufs=5))
        nfp = ctx.enter_context(tc.tile_pool(name="nfp", bufs=6))
        work = ctx.enter_context(tc.tile_pool(name="work", bufs=3))
        psum = ctx.enter_context(tc.tile_pool(name="psum", bufs=1, space="PSUM"))

        # --- small constants first on the DMA queue ---
        cstb = consts.tile([LR, NPOS * LR], f32r)
        nc.sync.dma_start(cstb, cstb_d)
        cstd = consts.tile([LR, NPOS * TH], f16)
        nc.sync.dma_start(cstd, cstd_d)
        cstv = consts.tile([128, 2 * NBLK], f32)
        nc.sync.dma_start(cstv, cstv_d)

        def gmat(p):
            return cstb[0:LR, p * LR : (p + 1) * LR]

        def dmat(p):
            return cstd[0:LR, p * TH : (p + 1) * TH]

        # --- on-chip builds (overlap the initial DMA window) ---
        # x-coordinate row (0..639) on every partition, f16 (exact ints)
        xio = consts.tile([128, W], i16)
        nc.gpsimd.iota(xio, [[1, W]], base=0, channel_multiplier=0)
        X16 = consts.tile([128, W], f16)
        nc.vector.tensor_copy(X16, xio)
        X = X16[0:TH, :]

        # identity mask: iota(i - p) == 0
        io16 = consts.tile([TH, TH], i16)
        nc.gpsimd.iota(io16, [[1, TH]], base=0, channel_multiplier=-1)
        mask = consts.tile([TH, TH], bf16)
        nc.vector.tensor_scalar(mask, io16, 0, None, OP.is_equal)
        mask32 = consts.tile([TH, TH], f32r)
        nc.vector.tensor_copy(mask32, mask)

        # diag/sid stationaries: image 0 built on DVE, image 1 DMA'd below
        dgt = consts.tile([TH, 2 * NBLK * TH], bf16)
        for blk in range(NBLK):
            nc.vector.tensor_scalar_mul(
                dgt[:, blk * TH : (blk + 1) * TH], mask, cstv[0:TH, blk : blk + 1]
            )

        def dg(i, p, k):
            blk = i * NBLK + p * 5 + k
            return dgt[0:TH, blk * TH : (blk + 1) * TH]

        def sid(i, s):
            blk = i * NBLK + 20 + s
            return dgt[0:TH, blk * TH : (blk + 1) * TH]

        acc = consts.tile([128, 1], f32)
        nc.vector.memset(acc, 0.0)
        ones_t = consts.tile([128, 1], f32)
        nc.vector.memset(ones_t, 1.0)

        rho_all = consts.tile([TH, NT * W], bf16)
        gelu_out = consts.tile([TH, NT * W], bf16)

        def front(t):
            i, p = divmod(t, NPOS)
            imgt = imgp.tile([LR, 3, W], f32r, tag="imgt")
            nc.sync.dma_start(
                imgt,
                img_d[i, :, RS[p] : RS[p] + LR, :].rearrange("c h w -> h c w"),
            )
            nft = nfp.tile([TH, 2, W], f32r, tag="nft")
            nc.sync.dma_start(
                nft,
                nf_d[i, :, TH * p : TH * (p + 1), :].rearrange("c h w -> h c w"),
            )
            return (imgt, nft)

        def grayblock(t, ft):
            imgt, nft = ft
            i, p = divmod(t, NPOS)
            # gray_ext (unscaled channel sum; halo rows at partitions 120/121)
            gray_ps = psum.tile([LR, 2, 512], f32, tag="gray")
            for c3 in range(3):
                for b, (n0, n1) in enumerate(NSPLIT):
                    nc.tensor.matmul(
                        gray_ps[:, b, 0:320],
                        gmat(p),
                        imgt[:, c3, n0:n1],
                        start=(c3 == 0),
                        stop=(c3 == 2),
                    )
            graypad = work.tile([LR, W + 2], f16, tag="graypad")
            nc.gpsimd.memset(graypad[:, 0:1], 0.0)
            nc.gpsimd.memset(graypad[:, W + 1 : W + 2], 0.0)
            nc.scalar.copy(half(graypad[:, 1 : W + 1]), gray_ps[:, :, 0:320])
            return (imgt, nft, graypad)

        def gymm(t, gt):
            imgt, nft, graypad = gt
            i, p = divmod(t, NPOS)
            # gy (row stencil) on PE off f16 graypad
            gy_ps = psum.tile([TH, 2, 512], f32, tag="gy")
            for b, (n0, n1) in enumerate(NSPLIT):
                nc.tensor.matmul(
                    gy_ps[:, b, 0:320],
                    dmat(p),
                    graypad[:, 1 + n0 : 1 + n1],
                    start=True,
                    stop=True,
                )
            return (t, i, p, nft, graypad, gy_ps)

        def stencil(sg):
            t, i, p, nft, graypad, gy_ps = sg
            # gy drain + gy^2 straight off PSUM on ACT (frees the gy banks)
            gyb = work.tile([TH, W], f16, tag="gyb")
            nc.scalar.copy(half(gyb), gy_ps[:, :, 0:320])
            gy2 = work.tile([TH, W], bf16, tag="gy2")
            nc.scalar.activation(half(gy2), gy_ps[:, :, 0:320], AF.Square)
            # gx: column-shift subtract + square on Pool
            gx = work.tile([TH, W], f16, tag="gx")
            nc.gpsimd.tensor_sub(
                gx, graypad[0:TH, 2 : W + 2], graypad[0:TH, 0:W]
            )
            gx2 = work.tile([TH, W], bf16, tag="gx2")
            nc.gpsimd.tensor_mul(gx2, gx, gx)
            return (t, i, p, nft, gx, gx2, gyb, gy2)

        def normblock(ns):
            t, i, p, nft, gx, gx2, gyb, gy2 = ns
            n2 = work.tile([TH, W], bf16, tag="n2")
            nc.vector.scalar_tensor_tensor(n2, gx2, EPS, gy2, OP.add, OP.add)
            rinv = work.tile([TH, W], bf16, tag="rinv")
            if USE_ARS:
                nc.scalar.activation(rinv, n2, AF.Abs_reciprocal_sqrt)
            else:
                n2f = work.tile([TH, W], f32, tag="n2f")
                nc.vector.tensor_copy(n2f, n2)
                inv2 = work.tile([TH, W], f32, tag="inv2")
                nc.vector.reciprocal_approx_fast(out=inv2, in_=n2f)
                nc.scalar.sqrt(rinv, inv2)
            return (t, i, p, nft, gx, gyb, rinv)

        def normprods(ns):
            t, i, p, nft, gx, gyb, rinv = ns
            gxn = work.tile([TH, W], bf16, tag="gxn")
            nc.vector.tensor_mul(gxn, gx, rinv)
            gyn = work.tile([TH, W], bf16, tag="gyn")
            nc.vector.tensor_mul(gyn, gyb, rinv)
            xgxn = work.tile([TH, W], bf16, tag="xgxn")
            nc.vector.tensor_mul(xgxn, gxn, X)
            xgyn = work.tile([TH, W], bf16, tag="xgyn")
            nc.vector.tensor_mul(xgyn, gyn, X)
            x2gxn = work.tile([TH, W], bf16, tag="x2gxn")
            nc.vector.tensor_mul(x2gxn, xgxn, X)
            return (t, i, p, nft, gxn, gyn, xgxn, xgyn, x2gxn)

        def backblock(st):
            t, i, p, nft, gxn, gyn, xgxn, xgyn, x2gxn = st
            # P/n = V2*xgxn - V0*gxn + (V2 y - V1) gyn
            P_ps = psum.tile([TH, 2, 512], f32, tag="P")
            psrc = [(sid(i, 0), xgxn), (sid(i, 1), gxn), (dg(i, p, 0), gyn)]
            for k, (m, src) in enumerate(psrc):
                for b, (n0, n1) in enumerate(NSPLIT):
                    nc.tensor.matmul(
                        P_ps[:, b, 0:320],
                        m,
                        src[:, n0:n1],
                        start=(k == 0),
                        stop=(k == len(psrc) - 1),
                    )
            # v = nf0 + nf1 - R/n
            #   = nf0 + nf1 + W1*x2gxn - W0y*xgxn + (W1y+W2)*xgyn
            #     + (W1 - W2y)*gxn - W0(y^2+1)*gyn
            v_ps = psum.tile([TH, 2, 512], f32, tag="V")
            vsrc = [
                (mask32, nft[:, 0, :]),
                (mask32, nft[:, 1, :]),
                (sid(i, 2), x2gxn),
                (dg(i, p, 1), xgxn),
                (dg(i, p, 2), xgyn),
                (dg(i, p, 3), gxn),
                (dg(i, p, 4), gyn),
            ]
            for k, (m, src) in enumerate(vsrc):
                for b, (n0, n1) in enumerate(NSPLIT):
                    nc.tensor.matmul(
                        v_ps[:, b, 0:320],
                        m,
                        src[:, n0:n1],
                        start=(k == 0),
                        stop=(k == len(vsrc) - 1),
                    )
            Pb = work.tile([TH, W], bf16, tag="pb")
            nc.scalar.copy(half(Pb), P_ps[:, :, 0:320])
            return (t, Pb, v_ps)

        def rhoblock(bk):
            t, Pb, v_ps = bk
            rho = rho_all[0:TH, t * W : (t + 1) * W]
            nc.vector.tensor_mul(half(rho), half(Pb), v_ps[:, :, 0:320])
            if check_mode:
                rho32 = work.tile([TH, W], f32, tag="rho32")
                nc.vector.tensor_copy(rho32, rho)
                nc.sync.dma_start(rho_d[:, t * W : (t + 1) * W], rho32)

        # --- software pipeline, 5-stage skew ---
        # iter k: rho(k) | P/v+Pb(k) | norms(k+1) | stencil(k+2) |
        #         gray(k+3) | dma(k+4); emission order makes each engine's
        #         in-order queue hit ops whose deps complete just-in-time.
        fts, gts, sgs, nss, sts, bks = {}, {}, {}, {}, {}, {}
        fts[0] = front(0)
        fts[1] = front(1)
        # image-1 stationaries arrive behind the first two tiles' data
        nc.sync.dma_start(dgt[:, NBLK * TH : 2 * NBLK * TH], dgt1_d)
        fts[2] = front(2)
        fts[3] = front(3)
        for k in range(-4, NT):
            if 0 <= k + 2 < NT and (k + 2) in gts:
                sgs[k + 2] = gymm(k + 2, gts.pop(k + 2))
            if 0 <= k + 1 < NT and (k + 1) in nss:
                nss[k + 1] = normblock(nss[k + 1])
            if 0 <= k < NT:
                bks[k] = backblock(sts.pop(k))
            if 0 <= k + 1 < NT and (k + 1) in nss:
                sts[k + 1] = normprods(nss.pop(k + 1))
            if 0 <= k < NT:
                rhoblock(bks.pop(k))
            if k + 4 < NT and (k + 4) not in fts:
                fts[k + 4] = front(k + 4)
            if 0 <= k + 2 < NT and (k + 2) in sgs:
                nss[k + 2] = stencil(sgs.pop(k + 2))
            if 0 <= k + 3 < NT:
                gts[k + 3] = grayblock(k + 3, fts.pop(k + 3))

        # --- tail: gelu + reduce ---
        if not check_mode:
            nc.scalar.activation(
                gelu_out[0:TH, :],
                rho_all[0:TH, :],
                AF.Gelu,
                scale=-1.0,
                accum_out=acc[0:TH, 0:1],
            )
        out_ps = psum.tile([1, 1], f32, tag="gray")
        nc.tensor.matmul(
            out_ps, acc[0:TH, 0:1], ones_t[0:TH, :], start=True, stop=True
        )
        res = consts.tile([1, 1], f32)
        nc.scalar.copy(res, out_ps)
        nc.sync.dma_start(out_d, res)

    nc.compile()
    return nc


def _host_constants(pose_np):
    """Host-built constants. cstb/cstd shared; cstv/dgt1 per core."""
    import ml_dtypes

    # gmat_ext: img row -> gray_ext partition (cols 0..119 = band rows,
    # col 120 = halo row band_start-1, col 121 = halo row band_start+120)
    gmat = np.zeros((LR, NPOS, LR), np.float32)
    # dmat: graypad partition -> gy row: gy[j] = gray[j+1] - gray[j-1]
    dmat = np.zeros((LR, NPOS, TH), np.float32)
    for p in range(NPOS):
        for j in range(LR):
            g = TH * p + j if j < TH else (TH * p - 1 if j == TH else TH * p + TH)
            if 0 <= g <= H - 1:
                gmat[g - RS[p], p, j] = 1.0
        for j in range(TH):
            q_plus = j + 1 if j < TH - 1 else LR - 1
            q_minus = j - 1 if j >= 1 else TH
            dmat[q_plus, p, j] += 1.0
            dmat[q_minus, p, j] -= 1.0
    cstb = gmat.reshape(LR, -1)
    cstd = dmat.reshape(LR, -1).astype(np.float16)

    cstv_list = []
    dgt1_list = []
    jj = np.arange(TH, dtype=np.float32)
    for core in range(NCORES):
        vals = np.zeros((128, 2 * NBLK), np.float32)
        for i in range(BPC):
            b = core * BPC + i
            V0, V1, V2, W0, W1, W2 = [float(x) for x in pose_np[b]]
            base = i * NBLK
            for p in range(NPOS):
                yv = TH * p + jj
                o = base + p * 5
                vals[0:TH, o + 0] = V2 * yv - V1
                vals[0:TH, o + 1] = -W0 * yv
                vals[0:TH, o + 2] = W1 * yv + W2
                vals[0:TH, o + 3] = W1 - W2 * yv
                vals[0:TH, o + 4] = -W0 * (yv * yv + 1.0)
            vals[0:TH, base + 20] = V2
            vals[0:TH, base + 21] = -V0
            vals[0:TH, base + 22] = W1
        cstv_list.append(vals)
        dgt1 = np.zeros((TH, NBLK * TH), np.float32)
        for blk in range(NBLK):
            np.fill_diagonal(
                dgt1[:, blk * TH : (blk + 1) * TH], vals[0:TH, NBLK + blk]
            )
        dgt1_list.append(dgt1.astype(ml_dtypes.bfloat16))
    return cstb, cstd, cstv_list, dgt1_list


def kernel(img_pair, pose, normal_flow):
    from concourse.bass_utils import run_bass_kernel_spmd

    img_pair = np.asarray(img_pair, dtype=np.float32)
    pose = np.asarray(pose, dtype=np.float32)
    normal_flow = np.asarray(normal_flow, dtype=np.float32)

    if "nc" not in _CACHE:
        _CACHE["nc"] = _build_program()
    nc = _CACHE["nc"]

    cstb, cstd, cstv_list, dgt1_list = _host_constants(pose)
    in_maps = []
    for core in range(NCORES):
        b0 = core * BPC
        in_maps.append(
            {
                "img3": np.ascontiguousarray(img_pair[b0 : b0 + BPC, :3]),
                "nf": np.ascontiguousarray(normal_flow[b0 : b0 + BPC]),
                "cstb": cstb,
                "cstd": cstd,
                "cstv": cstv_list[core],
                "dgt1": dgt1_list[core],
            }
        )

    _CACHE["in_maps"] = in_maps
    res = run_bass_kernel_spmd(nc, in_maps, core_ids=list(range(NCORES)))
    total = np.float64(0.0)
    for r in res.results:
        total += np.float64(r["out"][0, 0])
    out = np.float32(total / (B * H * W))
    return np.asarray(out, dtype=np.float32)
